# revision 1
# baseline (speedup 1.0000x reference)
"""AttentionDownSample Trainium2 kernel (8 NeuronCores, data-parallel over batch).

Reference computation (per batch element b):
  pooled = AvgPool2d(2)(fm)                        # [C, h, w]
  Q      = Wq @ pooled / sqrt(32)                  # [32, h, w]
  K_s    = Wk @ fm_s          (s = 2x2 window pos) # [32, h, w] x4
  logits = sum_r Q * K_s                           # [h, w, 4]
  attn   = softmax(logits, axis=-1)
  out    = sum_s fm_s * attn_s                     # [C, h, w]

Kernel strategy (per core, one batch element):
  * Qrep[32s+r, p] = Q[r, p]  via 4 PSUM-accumulated matmuls with weights
    WqT replicated x4 along free dim (folds the avg-pool into the PE).
  * Kstack[32s+r, p] = K_s[r, p] via 4 col-tiled matmuls (tile_position).
  * Mstack = Qrep * Kstack (one DVE mul); logits via block-ones reduce
    matmuls packed as [4j+s, pos] for the tile's 4 chunks so the softmax
    ops run on 16 partitions at once.
  * attn row broadcast over channels via one-hot-row selector matmuls;
    window-weighted sum U = sum_s Y_s via identity-weight PSUM-accumulating
    matmuls (the adds ride the TensorEngine instead of DVE).
All constant weight/selector matrices are precomputed on the host and passed
as extra DRAM parameters.
"""

import numpy as np
from contextlib import ExitStack

import concourse.bass as bass
import concourse.bacc as bacc_mod
import concourse.tile as tile
from concourse import mybir
from concourse.bass_utils import run_bass_kernel_spmd

F32 = mybir.dt.float32
BF16 = mybir.dt.bfloat16
AF = mybir.ActivationFunctionType

# problem dims (hardcoded; spec: fm [8,128,256,256], Wq/Wk [32,128])
B, C, H, W = 8, 128, 256, 256
PH, PW = H // 2, W // 2          # pooled 128 x 128
R = 32                           # reduce dim
QSCALE = 1.0 / (4.0 * np.sqrt(32.0))   # folds avgpool 1/4 and 1/sqrt(32)

RROWS = 32                       # raw rows per outer tile
CH = 512                         # positions per chunk (1 PSUM bank fp32)
NPACK = (RROWS // 2) * PW // CH  # chunks packed per tile (4)
# engine for the 4 Y_s = fm_s * attn_s multiplies: "dve" reads the broadcast
# attn from PSUM directly; "gps" needs an ACT copy of it into SBUF first.
MUL_ENGINE = ("dve", "dve", "gps", "gps")


def host_consts(Wq: np.ndarray, Wk: np.ndarray) -> dict:
    """Constant matrices computed host-side and DMA'd in once."""
    wqrep = np.tile(Wq.T.astype(np.float32) * QSCALE, (1, 4))        # [C, 128]
    wkT = np.ascontiguousarray(Wk.T.astype(np.float32))              # [C, 32]
    i128 = np.eye(C, dtype=np.float32)                               # [C, C]
    # bones packed [C, NPACK * 4*NPACK]: block j is a [C, 4*NPACK] matrix
    # whose col 4j+s has ones at rows 32s..32s+32 (zeros elsewhere, so each
    # chunk's matmul writes the full packed-logits tile).
    np4 = 4 * NPACK
    bones = np.zeros((C, NPACK * np4), dtype=np.float32)
    for j in range(NPACK):
        for s in range(4):
            bones[32 * s : 32 * s + 32, np4 * j + 4 * j + s] = 1.0
    # zsel [4*NPACK, NPACK]: zsel[4j+s, j] = 1
    zsel = np.zeros((4 * NPACK, NPACK), dtype=np.float32)
    # rsel [NPACK, 4*NPACK]: rsel[j, 4j+s] = 1
    rsel = np.zeros((NPACK, 4 * NPACK), dtype=np.float32)
    for j in range(NPACK):
        zsel[4 * j : 4 * j + 4, j] = 1.0
        rsel[j, 4 * j : 4 * j + 4] = 1.0
    # selw [4*NPACK, 4*NPACK * C]: block q ([*, C]) has row q all-ones
    selw = np.zeros((4 * NPACK, 4 * NPACK * C), dtype=np.float32)
    for q in range(4 * NPACK):
        selw[q, C * q : C * (q + 1)] = 1.0
    import ml_dtypes

    consts = {
        "wqrep": wqrep, "wkt": wkT, "i128": i128, "bones": bones,
        "zsel": zsel, "rsel": rsel, "selw": selw,
    }
    return {k: v.astype(ml_dtypes.bfloat16) for k, v in consts.items()}


def build_nc(h_rows: int = H) -> bass.Bass:
    """Build the SPMD single-core program. h_rows < H shrinks the image
    height (test/sim only)."""
    assert h_rows % RROWS == 0
    ntiles = h_rows // RROWS
    prows_t = RROWS // 2                      # pooled rows per tile (16)
    npos_t = prows_t * PW                     # pooled positions per tile (2048)
    assert NPACK == npos_t // CH
    crows = CH // PW                          # pooled rows per chunk (4)
    NP4 = 4 * NPACK

    nc = bacc_mod.Bacc(
        "TRN2", target_bir_lowering=False, debug=False, num_devices=B
    )
    fm = nc.declare_dram_parameter("fm", [C, h_rows, W], F32, isOutput=False)
    cwqrep = nc.declare_dram_parameter("wqrep", [C, C], BF16, isOutput=False)
    cwkt = nc.declare_dram_parameter("wkt", [C, R], BF16, isOutput=False)
    ci128 = nc.declare_dram_parameter("i128", [C, C], BF16, isOutput=False)
    cbones = nc.declare_dram_parameter("bones", [C, NPACK * NP4], BF16, isOutput=False)
    czsel = nc.declare_dram_parameter("zsel", [NP4, NPACK], BF16, isOutput=False)
    crsel = nc.declare_dram_parameter("rsel", [NPACK, NP4], BF16, isOutput=False)
    cselw = nc.declare_dram_parameter("selw", [NP4, NP4 * C], BF16, isOutput=False)
    out = nc.declare_dram_parameter("out", [C, h_rows // 2, PW], F32, isOutput=True)

    mm = nc.tensor.matmul

    with ExitStack() as ctx:
        tc = ctx.enter_context(tile.TileContext(nc))
        const = ctx.enter_context(tc.tile_pool(name="const", bufs=1))

        # ---- constants (DMA'd from host) -------------------------------
        wqrep = const.tile([C, C], BF16, tag="wqrep")
        nc.sync.dma_start(wqrep[:], cwqrep[:, :])
        wkT = const.tile([C, R], BF16, tag="wkT")
        nc.sync.dma_start(wkT[:], cwkt[:, :])
        i128 = const.tile([C, C], BF16, tag="i128")
        nc.sync.dma_start(i128[:], ci128[:, :])
        bones = const.tile([C, NPACK * NP4], BF16, tag="bones")
        nc.sync.dma_start(bones[:], cbones[:, :])
        zsel = const.tile([NP4, NPACK], BF16, tag="zsel")
        nc.sync.dma_start(zsel[:], czsel[:, :])
        rsel = const.tile([NPACK, NP4], BF16, tag="rsel")
        nc.sync.dma_start(rsel[:], crsel[:, :])
        selw = const.tile([NP4, NP4 * C], BF16, tag="selw")
        nc.sync.dma_start(selw[:], cselw[:, :])

        # ---- pools -----------------------------------------------------
        fmp = ctx.enter_context(tc.tile_pool(name="fmp", bufs=2))
        qrs = ctx.enter_context(tc.tile_pool(name="qrs", bufs=2))
        mst = ctx.enter_context(tc.tile_pool(name="mst", bufs=2))
        esb = ctx.enter_context(tc.tile_pool(name="esb", bufs=2))
        rsb = ctx.enter_context(tc.tile_pool(name="rsb", bufs=2))
        atn = ctx.enter_context(tc.tile_pool(name="atn", bufs=2))
        ecp = ctx.enter_context(tc.tile_pool(name="ecp", bufs=3))
        yp = ctx.enter_context(tc.tile_pool(name="yp", bufs=8))
        outp = ctx.enter_context(tc.tile_pool(name="outp", bufs=2))

        pq = ctx.enter_context(tc.tile_pool(name="pq", bufs=1, space="PSUM"))
        pk = ctx.enter_context(tc.tile_pool(name="pk", bufs=1, space="PSUM"))
        psm = ctx.enter_context(tc.tile_pool(name="psm", bufs=3, space="PSUM"))
        peb = ctx.enter_context(tc.tile_pool(name="peb", bufs=2, space="PSUM"))
        pu = ctx.enter_context(tc.tile_pool(name="pu", bufs=1, space="PSUM"))

        # ---- main loop -------------------------------------------------
        for t in range(ntiles):
            fm_t = fmp.tile([C, RROWS * W], BF16, tag="fm")
            nc.gpsimd.dma_start(
                fm_t[:],
                fm[:, RROWS * t : RROWS * (t + 1), :].rearrange("c h w -> c (h w)"),
            )
            # grid view: [c, i(pooled row), di, j(pooled col), dj]
            grid = fm_t[:].rearrange("c (i a j b) -> c i a j b", a=2, b=2, j=PW)

            def fview(s, j):
                di, dj = s >> 1, s & 1
                return grid[:, crows * j : crows * (j + 1), di, :, dj]

            out_sb = outp.tile([C, npos_t], F32, tag="out")
            lg_ps = psm.tile([NP4, CH], F32, tag="sm")

            # phase 1: per chunk -> packed logits
            for j in range(NPACK):
                qrep_ps = pq.tile([C, CH], F32, tag="pq")
                for s in range(4):
                    mm(
                        qrep_ps[:], wqrep[:], fview(s, j),
                        start=(s == 0), stop=(s == 3),
                    )
                kst_ps = pk.tile([C, CH], F32, tag="pk")
                for s in range(4):
                    mm(
                        kst_ps[32 * s : 32 * s + 32, :], wkT[:], fview(s, j),
                        start=True, stop=True, tile_position=(0, 32 * s),
                        skip_group_check=True,
                    )
                qrep_sb = qrs.tile([C, CH], BF16, tag="qr")
                nc.scalar.copy(qrep_sb[:], qrep_ps[:])
                m_sb = mst.tile([C, CH], BF16, tag="ms")
                nc.vector.tensor_mul(m_sb[:], qrep_sb[:], kst_ps[:])
                mm(
                    lg_ps[:], bones[:, NP4 * j : NP4 * (j + 1)], m_sb[:],
                    start=(j == 0), stop=(j == NPACK - 1), skip_group_check=True,
                )

            # phase 2: packed softmax for the whole tile
            e_sb = esb.tile([NP4, CH], BF16, tag="e")
            nc.scalar.activation(e_sb[:], lg_ps[:], AF.Exp)
            z_ps = psm.tile([NPACK, CH], F32, tag="sm")
            mm(z_ps[:], zsel[:], e_sb[:], start=True, stop=True)
            r_sb = rsb.tile([NPACK, CH], BF16, tag="r")
            with nc.allow_low_precision(reason="bf16 recip: attn scale, 2e-2 gate"):
                nc.vector.reciprocal(r_sb[:], z_ps[:])
            rb_ps = psm.tile([NP4, CH], F32, tag="sm")
            mm(rb_ps[:], rsel[:], r_sb[:], start=True, stop=True)
            at_sb = atn.tile([NP4, CH], BF16, tag="at")
            nc.vector.tensor_mul(at_sb[:], e_sb[:], rb_ps[:])

            # phase 3: per chunk -> attn broadcast, Y muls, U accumulation
            for j in range(NPACK):
                ys = []
                for s in range(4):
                    q = 4 * j + s
                    e_ps = peb.tile([C, CH], F32, tag="eb")
                    mm(
                        e_ps[:], selw[:, C * q : C * (q + 1)], at_sb[:],
                        start=True, stop=True,
                    )
                    y = yp.tile([C, CH], BF16, tag="y")
                    yv = y[:].rearrange("c (i j) -> c i j", j=PW)
                    if MUL_ENGINE[s] == "dve":
                        nc.vector.tensor_mul(
                            yv, fview(s, j),
                            e_ps[:].rearrange("c (i j) -> c i j", j=PW),
                        )
                    else:
                        e_cp = ecp.tile([C, CH], BF16, tag="ec")
                        nc.scalar.copy(e_cp[:], e_ps[:])
                        nc.gpsimd.tensor_mul(
                            yv, fview(s, j),
                            e_cp[:].rearrange("c (i j) -> c i j", j=PW),
                        )
                    ys.append(y)
                u_ps = pu.tile([C, CH], F32, tag="pu")
                for s in range(4):
                    mm(
                        u_ps[:], i128[:], ys[s][:], start=(s == 0), stop=(s == 3)
                    )
                nc.scalar.copy(out_sb[:, CH * j : CH * (j + 1)], u_ps[:])

            nc.sync.dma_start(
                out[:, prows_t * t : prows_t * (t + 1), :].rearrange(
                    "c h w -> c (h w)"
                ),
                out_sb[:],
            )

    nc.compile()
    return nc


_CACHE: dict = {}


def _get_nc(h_rows: int = H) -> bass.Bass:
    if h_rows not in _CACHE:
        _CACHE[h_rows] = build_nc(h_rows)
    return _CACHE[h_rows]


def kernel(fm: np.ndarray, Wq: np.ndarray, Wk: np.ndarray, **run_kwargs) -> np.ndarray:
    assert fm.shape == (B, C, H, W), fm.shape
    nc = _get_nc(H)
    consts = host_consts(Wq, Wk)
    in_maps = [
        {"fm": np.ascontiguousarray(fm[b], dtype=np.float32), **consts}
        for b in range(B)
    ]
    res = run_bass_kernel_spmd(nc, in_maps, core_ids=list(range(B)), **run_kwargs)
    out = np.stack([res.results[b]["out"] for b in range(B)], axis=0)
    kernel.last_result = res
    return out


kernel.last_result = None

